# revision 22
# baseline (speedup 1.0000x reference)
"""Trainium2 Bass kernel for nn_Attention_13073880449373.

Full-batch multi-head attention (B=8, S=1024, C=1024, H=16, D=64) with RoPE,
data-parallel over the batch dim: core b computes batch b end-to-end.

v4: whole-kernel software pipeline balanced against the ACT engine.
The exp() of the attention scores is ~142us of ACT work (16.7M elems at
~118 elem/ns) vs ~194us of PE work; v2 interleaved scores and pv of the
SAME head pair only 2 k-chunks apart, so pv stalled on exp whenever no
qk GEMM work was interleaved (pure-attention windows ran ~2x slow), and
the qk GEMM shared the 2-slot score PSUM rotation, so each 16-slot qk
burst starved the ACT engine. v3:
  - pipelines at pair granularity: iteration p runs scores(p), pv(p-1),
    and the qk GEMM for pair p+2 (two-iteration lookahead hides the
    whole RoPE chain), giving every exp a full iteration of slack.
  - PSUM (8 banks): 3 rotating [128,1024] score slots (6 banks) keep
    the ACT fed through the 8-slot qk/pv lumps; qk/v accumulate in one
    [128,512] slot; pv accumulates per (head, n-half) "quarter" through
    one [65,512] slot, evacuating each quarter mid-iteration so the
    next pair never waits on the normalize chain.
  - v-bias and proj-bias folded out of the PE: softmax rows sum to 1 so
    attn(v + b_v) = attn(v) + b_v, hence b_eff = b_v @ W_proj + b_proj
    is added during the output evacuation, saving 32 matmul slots.
  - weight DMAs issue from the gpsimd queue (the ACT queue issued 26
    DMAs = 16us in v2); Wv DMAs defer behind the prologue rotate-DMAs so
    they don't steal startup bandwidth from xk/Wqk; qk halves spread
    4/4/4/4/2/2 over iters 1-6 so the ACT-paced late iterations keep PE
    work; denominators ride as oS row 64 (one copy frees the pv bank),
    reciprocal_approx_fast off a sync-DMA-staged row, normalize mults
    placed at sk7 where their gpsimd-broadcast wait cannot block evac
    copies; epilogue normalize uses PE outer-product broadcasts and the
    output stages through bf16 (host upcasts).

Probed hardware quirks this build works around: partition_broadcast and
the custom-DVE reciprocal ops silently mishandle APs with nonzero
partition offsets (stage through partition 0); gpsimd cannot access
PSUM; DVE ops need 32-aligned partition starts; gpsimd tensor ops run
~5-10x slower than DVE.
"""

import math
import os
from contextlib import ExitStack

import numpy as np

B, S, C = 8, 1024, 1024
H, D = 16, 64
N_CORES = 8
KC = C // 128  # 8 contraction chunks of 128
NPAIR = H // 2

_CACHE = {}


def _cs_table():
    # Matches reference.rope_cos_sin computed in float32, transposed, with the
    # rotate-half sign folded into the sin half (rows 0-31 negated).
    f = np.float32
    inv = np.exp(np.arange(0, D, 2, dtype=f) * f(-(math.log(10000.0) / D))).astype(f)
    pos = np.arange(S, dtype=f)[:, None]
    ang = (pos * inv[None, :]).astype(f)  # (S, 32)
    ang = np.concatenate([ang, ang], axis=1)  # (S, 64)
    cosT = np.cos(ang).T.astype(f)  # (64, S)
    sinT = np.sin(ang).T.astype(f)
    sign = np.where(np.arange(D) < D // 2, f(-1.0), f(1.0))[:, None].astype(f)
    half = np.concatenate([cosT, sinT * sign], axis=1)  # (64, 2S)
    return np.concatenate([half, half], axis=0).astype(f)  # (128, 2S)


def declare_io(nc):
    from concourse import mybir

    f32 = mybir.dt.float32
    bf16 = mybir.dt.bfloat16
    return {
        "xT": nc.dram_tensor("xT", [C, S], bf16, kind="ExternalInput").ap(),
        "Wqk": nc.dram_tensor("Wqk", [C, 2 * C], bf16, kind="ExternalInput").ap(),
        "bqk": nc.dram_tensor("bqk", [128, 16], f32, kind="ExternalInput").ap(),
        "Wv": nc.dram_tensor("Wv", [C, C], bf16, kind="ExternalInput").ap(),
        "Wp": nc.dram_tensor("Wp", [C, C], bf16, kind="ExternalInput").ap(),
        "cs": nc.dram_tensor("cs", [128, 2 * S], bf16, kind="ExternalInput").ap(),
        "beff": nc.dram_tensor("beff", [1, C], f32, kind="ExternalInput").ap(),
        "out": nc.dram_tensor("out", [S, C], bf16, kind="ExternalOutput").ap(),
    }


def _emit(tc, io=None):
    from concourse import mybir
    from concourse.bass import ds, ts

    nc = tc.nc
    f32 = mybir.dt.float32
    bf16 = mybir.dt.bfloat16
    AF = mybir.ActivationFunctionType
    MUL = mybir.AluOpType.mult
    ADD = mybir.AluOpType.add

    if io is None:
        io = declare_io(nc)
    xT = io["xT"]
    Wqk = io["Wqk"]
    bqk = io["bqk"]
    Wv = io["Wv"]
    Wp = io["Wp"]
    cs = io["cs"]
    beff = io["beff"]
    out = io["out"]

    with ExitStack() as ctx:
        # ---------------- long-lived consts (right side) ----------------
        kons = ctx.enter_context(tc.tile_pool(name="kons", bufs=1, side="right"))
        cs_t = kons.tile([128, 2 * S], bf16, name="cs_t")
        bqk2 = kons.tile([128, 16], f32, name="bqk2")
        beff_r = kons.tile([1, C], f32, name="beff_r")
        beff_b = kons.tile([128, C], f32, name="beff_b")
        ones64 = kons.tile([1, 64], f32, name="ones64")
        nc.vector.memset(ones64[:], 1.0)

        # PSUM (8 banks): "sc" 3x[128,1024] (6) + "sm" 1x[128,512] (1)
        # + "pv" 1x[65,512] (1)
        ps_p = ctx.enter_context(tc.tile_pool(name="ps", bufs=1, space="PSUM"))

        def ps_sc(name):
            return ps_p.tile([128, S], f32, name=name, tag="sc", bufs=3)

        def ps_half(name, tag):
            return ps_p.tile([128, 512], f32, name=name, tag=tag, bufs=1)

        # the two 1-bank slots serve qk halves, v halves, and pv quarters
        # in strict emission-order alternation, so each bank gets ~2 lumps
        # of slack before its WAR reuse (rides out DVE queue delay on the
        # evac copies)
        _smp = {"i": 0}

        def next_small_tag():
            _smp["i"] ^= 1
            return "pv" if _smp["i"] else "sm"

        # ---------------- SBUF pools ----------------
        xk_p = ctx.enter_context(tc.tile_pool(name="xk", bufs=8))
        wqk_p = ctx.enter_context(tc.tile_pool(name="wqk", bufs=2))
        rr_p = ctx.enter_context(tc.tile_pool(name="rr", bufs=2))
        tm_p = ctx.enter_context(tc.tile_pool(name="tm", bufs=2))
        qkr_p = ctx.enter_context(tc.tile_pool(name="qkr", bufs=6))
        vst_p = ctx.enter_context(tc.tile_pool(name="vst", bufs=8))
        wv_p = ctx.enter_context(tc.tile_pool(name="wv", bufs=8))
        pT_p = ctx.enter_context(tc.tile_pool(name="pT", bufs=24))
        rec_p = ctx.enter_context(tc.tile_pool(name="rec", bufs=1))
        rb_p = ctx.enter_context(tc.tile_pool(name="rb", bufs=1))
        oS_p = ctx.enter_context(tc.tile_pool(name="oS", bufs=2))
        aT_p = ctx.enter_context(tc.tile_pool(name="aT", bufs=8))

        # -------- input DMAs: weights on gpsimd queue, rest on sync --------
        def qk_pair_weights(pair):
            # two [128,1024] half-DMAs, q-half first: the first qk matmul
            # then waits on 256KB instead of the full pair tile
            w = wqk_p.tile([128, 8 * 256], bf16, name=f"wqk{pair}", tag="wqk")
            for a in range(2):
                nc.gpsimd.dma_start(
                    out=w[:, ds(a * 1024, 1024)],
                    in_=Wqk[ts(pair, 128), ds(a * 1024, 1024)],
                )
            return w

        w_d = {0: qk_pair_weights(0)}
        xk = []
        for k in range(KC):
            t = xk_p.tile([128, S], bf16, name=f"xk{k}", tag="xk")
            xk.append(t)
            nc.sync.dma_start(out=t[:], in_=xT[ts(k, 128), :])
        nc.sync.dma_start(out=cs_t[:], in_=cs[:])
        nc.sync.dma_start(out=bqk2[:], in_=bqk[:])
        nc.sync.dma_start(out=beff_r[:], in_=beff[:])
        w_d[1] = qk_pair_weights(1)

        # qk GEMM in two [128,512] half-passes + RoPE chain
        rr_d = {}

        def qk_half(pair, a, n, ptag=None):
            gm = a * 8 + pair
            if (pair, a) not in rr_d:
                rr_d[(pair, a)] = rr_p.tile(
                    [128, 2 * S], bf16, name=f"rr{gm}", tag="rr"
                )
            rr = rr_d[(pair, a)]
            ps = ps_half(f"qps{gm}_{n}", next_small_tag())
            wts = w_d[pair]
            for k in range(KC):
                w = wts[:, a * 1024 + k * 128 : a * 1024 + k * 128 + 128]
                nc.tensor.matmul(
                    ps[:],
                    w,
                    xk[k][:, ds(n * 512, 512)],
                    start=(k == 0),
                    stop=(k == KC - 1),
                )
            nc.vector.tensor_scalar_add(
                rr[:, ds(n * 512, 512)], ps[:], bqk2[:, gm : gm + 1]
            )

        def qk_finish(pair, a):
            """rotate-half + cos/sin multiply-add -> RoPE'd qT or kT."""
            gm = a * 8 + pair
            rr = rr_d.pop((pair, a))
            for d0, s0 in ((0, 32), (32, 0), (64, 96), (96, 64)):
                nc.sync.dma_start(
                    out=rr[d0 : d0 + 32, S : 2 * S], in_=rr[s0 : s0 + 32, 0:S]
                )
            tm = tm_p.tile([128, 2 * S], bf16, name=f"tm{gm}", tag="tm")
            for half in range(2):
                nc.vector.tensor_tensor(
                    tm[:, ds(half * S, S)],
                    rr[:, ds(half * S, S)],
                    cs_t[:, ds(half * S, S)],
                    MUL,
                )
            qt = qkr_p.tile([128, S], bf16, name=f"qkr{gm}", tag="qkr")
            nc.vector.tensor_tensor(qt[:], tm[:, 0:S], tm[:, S : 2 * S], ADD)
            return qt

        # ---------------- v phase ----------------
        # wv tiles are declared here but their DMAs are issued after the
        # prologue qk emission on the sync queue: behind the rotate-half
        # DMAs (which wait on the RoPE chain), so the 2MB of Wv traffic
        # defers until the startup-critical xk/w0/w1 transfers are done.
        wv = []
        for k in range(KC):
            t = wv_p.tile([128, C], bf16, name=f"wv{k}", tag="wv")
            wv.append(t)
        vst = []

        def v_tile(mv):
            t = vst_p.tile([128, H * 65], bf16, name=f"vst{mv}", tag="vst")
            vst.append(t)
            hv = t[:].rearrange("p (h u) -> p h u", u=65)
            nc.vector.memset(hv[:, :, 64:65], 1.0)
            return hv

        def v_half(mv, n, hv, ptag=None):
            ps = ps_half(f"vps{mv}_{n}", next_small_tag())
            for k in range(KC):
                nc.tensor.matmul(
                    ps[:],
                    xk[k][:, ts(mv, 128)],
                    wv[k][:, ds(n * 512, 512)],
                    start=(k == 0),
                    stop=(k == KC - 1),
                )
            # channels n*512..: heads n*8..n*8+7 (head h = channel // 64).
            # Evac on ACT: it is idle in iter 0 while the DVE runs the qk
            # RoPE chains.
            nc.scalar.activation(hv[:, ds(n * 8, 8), 0:64], ps[:], AF.Copy)

        # ---------------- attention helpers ----------------
        def sc_step(pair, sk, qt, kt, pTs):
            """Scores + exp for both heads of the pair at k-chunk sk.
            Even/odd heads sit on PE row groups 0-63 / 64-127 and co-run."""
            heads = (2 * pair, 2 * pair + 1)
            sc = {}
            for h in heads:
                pTs[(h, sk)] = pT_p.tile(
                    [128, S], bf16, name=f"pT{h}_{sk}", tag="pT"
                )
                sc[h] = ps_sc(f"sc{h}_{sk}")
            for n in range(2):
                for h in heads:
                    u = h % 2
                    nc.tensor.matmul(
                        sc[h][:, ds(n * 512, 512)],
                        kt[ds(64 * u, 64), ts(sk, 128)],
                        qt[ds(64 * u, 64), ds(n * 512, 512)],
                        start=True,
                        stop=True,
                    )
            for h in heads:
                nc.scalar.activation(pTs[(h, sk)][:], sc[h][:], AF.Exp, scale=0.125)

        def pv_quarter(pair, u, n, pTs, oS):
            """All 8 k-chunks of one (head-parity, n-half) quarter through
            one pv PSUM bank; a single [65,512] copy (row 64 = softmax
            denominator) releases the bank."""
            h = 2 * pair + u
            ps = ps_p.tile(
                [65, 512], f32, name=f"pv{h}_{n}", tag=next_small_tag(), bufs=1
            )
            for sk in range(KC):
                nc.tensor.matmul(
                    ps[:],
                    vst[sk][:, 65 * h : 65 * h + 65],
                    pTs[(h, sk)][:, ds(n * 512, 512)],
                    start=(sk == 0),
                    stop=(sk == KC - 1),
                )
            nc.vector.tensor_copy(oS[u][0:65, ds(n * 512, 512)], ps[0:65, :])

        # finish_pair is split in two emission phases so the DVE mults never
        # sit in the queue waiting on the (slow, ~2us) gpsimd broadcasts and
        # block the latency-critical pv-bank evac copies behind them.
        # reciprocal_approx_fast (~18 bits, plenty for softmax denominators)
        # reads the denominator row straight out of oS and writes partition
        # 0, where partition_broadcast needs it.
        def finish_pair_a(pair, oS):
            rbs = {}
            for u in range(2):
                # custom-DVE ops (like partition_broadcast) mishandle nonzero
                # partition offsets, so the denominator row is staged to
                # partition 0 by a sync-queue DMA (off the busy DVE)
                dn = rec_p.tile([1, S], f32, name=f"dn{pair}_{u}", tag=f"dn{u}")
                nc.sync.dma_start(out=dn[0:1, :], in_=oS[u][64:65, :])
                rec = rec_p.tile([1, S], f32, name=f"rec{pair}_{u}", tag=f"rc{u}")
                for nh in range(2):
                    nc.vector.reciprocal_approx_fast(
                        out=rec[0:1, ds(nh * 512, 512)],
                        in_=dn[0:1, ds(nh * 512, 512)],
                    )
                rb = rb_p.tile([64, S], f32, name=f"rb{pair}_{u}", tag=f"rb{u}")
                nc.gpsimd.partition_broadcast(rb[:], rec[0:1, :])
                rbs[u] = rb
            return rbs

        def finish_pair_b(pair, oS, rbs):
            aT_t = aT_p.tile([128, S], bf16, name=f"aT{pair}", tag="aT")
            for u in range(2):
                for nh in range(2):
                    nc.vector.tensor_tensor(
                        aT_t[ds(64 * u, 64), ds(nh * 512, 512)],
                        oS[u][0:64, ds(nh * 512, 512)],
                        rbs[u][:, ds(nh * 512, 512)],
                        MUL,
                    )
            return aT_t

        aT = []
        qk_done = {}
        QORDER = [(0, 0), (0, 1), (1, 0), (1, 1)]  # (head parity, n-half)

        # ================= prologue: qk(0), qk(1) =================
        for pair in (0, 1):
            for a in (0, 1):
                qk_half(pair, a, 0, "sm")
                qk_half(pair, a, 1, "pv")
                qk_done.setdefault(pair, []).append(qk_finish(pair, a))
        nc.gpsimd.partition_broadcast(beff_b[:], beff_r[0:1, :])
        for k in range(KC):
            nc.sync.dma_start(out=wv[k][:], in_=Wv[ts(k, 128), :])

        # ================= iter 0: scores(0) + v GEMM + qk(2) ============
        pTs_prev = {}
        w_d[2] = qk_pair_weights(2)
        for sk in range(KC):
            sc_step(0, sk, qk_done[0][0], qk_done[0][1], pTs_prev)
            hv = v_tile(sk)
            v_half(sk, 0, hv, "sm")
            v_half(sk, 1, hv, "pv")
            if sk % 2 == 1:
                idx = sk // 2  # 0..3 -> (a, n)
                a, n = idx // 2, idx % 2
                qk_half(2, a, n, "sm")
                if n == 1:
                    qk_done.setdefault(2, []).append(qk_finish(2, a))
        w_d[3] = qk_pair_weights(3)

        den_d, oS_d, rbs_d, aTq = {}, {}, {}, []

        # ================= steady iterations p = 1..7 =================
        # iteration p: scores(p) + pv quarters (p-1) + qk halves (p+2)
        for p in range(1, NPAIR):
            pTs = {}
            qt, kt = qk_done[p]
            oS = {
                u: oS_p.tile([65, S], f32, name=f"oS{p - 1}_{u}", tag=f"oS{u}")
                for u in range(2)
            }
            # qk halves for pairs 3..7 spread 4/4/4/4/2/2 over iters 1..6
            # (pair 7's kt finishes RoPE just before iter 7 consumes it), so
            # the late iterations keep PE work while the ACT catches up on
            # exp; halves sit at sk 0,2,4,7 and pv quarters at 1,3,5,6 (the
            # last quarter must not sit adjacent to the next iteration's
            # first quarter in the 2-bank rotation)
            qk_sched = {
                1: [0, 1, 2, 3], 2: [4, 5, 6, 7], 3: [8, 9, 10, 11],
                4: [12, 13, 14, 15], 5: [16, 17], 6: [18, 19],
            }
            sk_pos = (0, 2, 4, 7)
            qk_at = {
                sk_pos[i]: g for i, g in enumerate(qk_sched.get(p, []))
            }
            pv_at = {1: 0, 3: 1, 5: 2, 6: 3}
            for sk in range(KC):
                sc_step(p, sk, qt, kt, pTs)
                if sk in qk_at:
                    g = qk_at[sk]
                    q, idx = 3 + g // 4, g % 4
                    a, n = idx // 2, idx % 2
                    qk_half(q, a, n)
                    if n == 1:
                        qk_done.setdefault(q, []).append(qk_finish(q, a))
                if sk in pv_at:
                    u, n = QORDER[pv_at[sk]]
                    pv_quarter(p - 1, u, n, pTs_prev, oS)
                if sk == 3 and aTq:
                    pp2 = aTq[0]  # normalize pair p-2 (evac'd last iter)
                    rbs_d[pp2] = finish_pair_a(pp2, oS_d[pp2])
                if sk == 7 and aTq:
                    pp2 = aTq.pop(0)
                    aT.append(finish_pair_b(pp2, oS_d[pp2], rbs_d.pop(pp2)))
            oS_d[p - 1] = oS
            aTq.append(p - 1)
            if p + 3 < NPAIR:
                w_d[p + 3] = qk_pair_weights(p + 3)
            if p == 4:
                # proj weights reuse the wv slots (v GEMM done in iter 0)
                wp = []
                for k in range(KC):
                    t = wv_p.tile([128, C], bf16, name=f"wp{k}", tag="wv")
                    nc.gpsimd.dma_start(out=t[:], in_=Wp[ts(k, 128), :])
                    wp.append(t)
            pTs_prev = pTs

        # ================= epilogue: pv(7) + projection =================
        oS7 = {
            u: oS_p.tile([65, S], f32, name=f"oS7_{u}", tag=f"oS{u}")
            for u in range(2)
        }
        p6 = aTq.pop(0)
        aT6 = aT_p.tile([128, S], bf16, name="aT6", tag="aT")
        recE = {
            (pr, u): rec_p.tile([1, S], f32, name=f"recE{pr}_{u}", tag=f"rc{u}")
            for pr in (6, 7)
            for u in range(2)
        }

        def finishE(pr, oS, aT_t, nh):
            """PE outer-product normalize for one n-half (epilogue pairs:
            the small PSUM banks are free and the chain must be short)."""
            h = ds(nh * 512, 512)
            for u in range(2):
                dn = rec_p.tile(
                    [1, S], f32, name=f"dnE{pr}_{u}_{nh}", tag=f"dn{u}"
                )
                nc.sync.dma_start(out=dn[0:1, h], in_=oS[u][64:65, h])
                rec = recE[(pr, u)]
                nc.vector.reciprocal_approx_fast(out=rec[0:1, h], in_=dn[0:1, h])
                rbp = ps_p.tile(
                    [64, 512], f32, name=f"rbE{u}_{nh}", tag=next_small_tag(), bufs=1
                )
                nc.tensor.matmul(rbp[:], ones64[:], rec[0:1, h], start=True, stop=True)
                nc.vector.tensor_tensor(
                    aT_t[ds(64 * u, 64), h], oS[u][0:64, h], rbp[:], MUL
                )

        finishE(p6, oS_d[p6], aT6, 0)
        aT.append(aT6)  # aT[6]

        def proj_open(m):
            # contracts pairs 0..6 (everything that does not depend on the
            # last pair's aT) into an sc slot
            pp = ps_sc(f"pp{m}")
            for k in range(KC - 1):
                for n in range(2):
                    nc.tensor.matmul(
                        pp[:, ds(n * 512, 512)],
                        aT[k][:, ts(m, 128)],
                        wp[k][:, ds(n * 512, 512)],
                        start=(k == 0),
                        stop=False,
                    )
            return pp

        def proj_close(m, pp):
            k = KC - 1
            for n in range(2):
                nc.tensor.matmul(
                    pp[:, ds(n * 512, 512)],
                    aT[k][:, ts(m, 128)],
                    wp[k][:, ds(n * 512, 512)],
                    start=False,
                    stop=True,
                )
            # bias add (b_v @ W_proj + b_proj) during the PSUM evac; the ob
            # staging tile reuses a wqk slot (qk weights are done)
            # bf16 staging halves the output DMA (the host upcasts); the
            # rounding adds ~0.2% rel err, well inside the gate
            ob = wqk_p.tile([128, C], bf16, name=f"ob{m}", tag="wqk")
            nc.vector.tensor_tensor(ob[:], pp[:], beff_b[:], ADD)
            q = nc.sync if m % 2 == 0 else nc.gpsimd
            q.dma_start(out=out[ts(m, 128), :], in_=ob[:])

        # pair-7 finish is pipelined per n-half: the rb broadcast runs as a
        # PE outer product (ones64 x rec-row) into the free sm/pv PSUM
        # slots, and closes 0-3 need only the n0 half of aT7, so the
        # projection never waits on the full normalize chain.
        aT7 = aT_p.tile([128, S], bf16, name="aT7", tag="aT")

        pv_quarter(NPAIR - 1, 0, 0, pTs_prev, oS7)
        finishE(p6, oS_d[p6], aT6, 1)
        pv_quarter(NPAIR - 1, 0, 1, pTs_prev, oS7)
        pps = {0: proj_open(0)}
        pv_quarter(NPAIR - 1, 1, 0, pTs_prev, oS7)
        pps[1] = proj_open(1)
        pv_quarter(NPAIR - 1, 1, 1, pTs_prev, oS7)
        finishE(NPAIR - 1, oS7, aT7, 0)
        aT.append(aT7)  # aT[7]
        pps[2] = proj_open(2)
        finishE(NPAIR - 1, oS7, aT7, 1)
        for m in range(3, S // 128):
            proj_close(m - 3, pps.pop(m - 3))
            pps[m] = proj_open(m)
        for m in range(S // 128 - 3, S // 128):
            proj_close(m, pps.pop(m))


def build_program():
    """Build + compile the Bass program (cached)."""
    if "nc" in _CACHE:
        return _CACHE["nc"]
    import concourse.tile as tile
    from concourse import bacc

    nc = bacc.Bacc(
        "TRN2", target_bir_lowering=False, debug=False, num_devices=N_CORES
    )
    with tile.TileContext(nc) as tc:
        _emit(tc)
    nc.compile()
    _CACHE["nc"] = nc
    return nc


def host_inputs(x, W_qkv, b_qkv, W_proj, b_proj):
    """Per-core input maps (host-side shard + layout prep)."""
    import ml_dtypes

    f = np.float32
    bf = ml_dtypes.bfloat16
    x = np.asarray(x, dtype=f)
    W_qkv = np.asarray(W_qkv, dtype=f)
    b_qkv = np.asarray(b_qkv, dtype=f)
    W_proj = np.asarray(W_proj, dtype=f)
    b_proj = np.asarray(b_proj, dtype=f)
    Wq4 = W_qkv[:, : 2 * C].reshape(8, 128, 2, 8, 128)  # (k, p, a, g, c)
    Wqk = np.ascontiguousarray(
        Wq4.transpose(3, 1, 2, 0, 4).reshape(8 * 128, 2 * C)
    ).astype(bf)  # (g*128+p, a*1024+k*128+c)
    bqk = np.ascontiguousarray(b_qkv[: 2 * C].reshape(16, 128).T).astype(f)
    Wv = W_qkv[:, 2 * C :].astype(bf)
    Wp = W_proj.astype(bf)
    # softmax rows sum to 1: attn(v + b_v) = attn(v) + b_v, so the v bias
    # rides through attention and folds into the proj bias.
    beff = (b_qkv[None, 2 * C :] @ W_proj + b_proj[None, :]).astype(f)
    cs = _cs_table().astype(bf)
    maps = []
    for b in range(B):
        maps.append(
            {
                "xT": np.ascontiguousarray(x[b].T).astype(bf),
                "Wqk": Wqk,
                "bqk": bqk,
                "Wv": Wv,
                "Wp": Wp,
                "cs": cs,
                "beff": beff,
            }
        )
    return maps


def _install_neff_cache():
    """Memoize the BIR->NEFF compile so repeat kernel() calls skip the
    multi-minute neuronxcc invocation (pure caching, same artifacts)."""
    if _CACHE.get("neff_cache"):
        return
    import hashlib
    import shutil
    import tempfile

    import concourse.bass2jax as b2j
    import concourse.bass_utils as bu

    cache_dir = os.path.join(tempfile.gettempdir(), "bass_neff_cache")
    os.makedirs(cache_dir, exist_ok=True)
    orig = bu.compile_bir_kernel

    def cached(bir_json, tmpdir, neff_name="file.neff"):
        raw = bir_json if isinstance(bir_json, bytes) else bir_json.encode()
        hit = os.path.join(cache_dir, hashlib.sha256(raw).hexdigest() + ".neff")
        if os.path.exists(hit):
            dst = os.path.join(tmpdir, neff_name)
            shutil.copyfile(hit, dst)
            return dst
        path = orig(bir_json, tmpdir, neff_name)
        try:
            shutil.copyfile(path, hit)
        except OSError:
            pass
        return path

    bu.compile_bir_kernel = cached
    b2j.compile_bir_kernel = cached
    _CACHE["neff_cache"] = True


def kernel(x, W_qkv, b_qkv, W_proj, b_proj):
    from concourse.bass_utils import run_bass_kernel_spmd

    _install_neff_cache()
    nc = build_program()
    in_maps = host_inputs(x, W_qkv, b_qkv, W_proj, b_proj)
    res = run_bass_kernel_spmd(nc, in_maps, list(range(N_CORES)))
    return np.stack(
        [np.asarray(r["out"]) for r in res.results], axis=0
    ).astype(np.float32)


if __name__ == "__main__":
    nc = build_program()
    print("program built + compiled OK")


# revision 23
# speedup vs baseline: 1.0363x; 1.0363x over previous
"""Trainium2 Bass kernel for nn_Attention_13073880449373.

Full-batch multi-head attention (B=8, S=1024, C=1024, H=16, D=64) with RoPE,
data-parallel over the batch dim: core b computes batch b end-to-end.

v4: whole-kernel software pipeline balanced against the ACT engine.
The exp() of the attention scores is ~142us of ACT work (16.7M elems at
~118 elem/ns) vs ~194us of PE work; v2 interleaved scores and pv of the
SAME head pair only 2 k-chunks apart, so pv stalled on exp whenever no
qk GEMM work was interleaved (pure-attention windows ran ~2x slow), and
the qk GEMM shared the 2-slot score PSUM rotation, so each 16-slot qk
burst starved the ACT engine. v3:
  - pipelines at pair granularity: iteration p runs scores(p), pv(p-1),
    and the qk GEMM for pair p+2 (two-iteration lookahead hides the
    whole RoPE chain), giving every exp a full iteration of slack.
  - PSUM (8 banks): 3 rotating [128,1024] score slots (6 banks) keep
    the ACT fed through the 8-slot qk/pv lumps; qk/v accumulate in one
    [128,512] slot; pv accumulates per (head, n-half) "quarter" through
    one [65,512] slot, evacuating each quarter mid-iteration so the
    next pair never waits on the normalize chain.
  - v-bias and proj-bias folded out of the PE: softmax rows sum to 1 so
    attn(v + b_v) = attn(v) + b_v, hence b_eff = b_v @ W_proj + b_proj
    is added during the output evacuation, saving 32 matmul slots.
  - weight DMAs issue from the gpsimd queue (the ACT queue issued 26
    DMAs = 16us in v2); Wv DMAs defer behind the prologue rotate-DMAs so
    they don't steal startup bandwidth from xk/Wqk; qk halves spread
    4/4/4/4/2/2 over iters 1-6 so the ACT-paced late iterations keep PE
    work; denominators ride as oS row 64 (one copy frees the pv bank),
    reciprocal_approx_fast off a sync-DMA-staged row, normalize mults
    placed at sk7 where their gpsimd-broadcast wait cannot block evac
    copies; epilogue normalize uses PE outer-product broadcasts and the
    output stages through bf16 (host upcasts).

Probed hardware quirks this build works around: partition_broadcast and
the custom-DVE reciprocal ops silently mishandle APs with nonzero
partition offsets (stage through partition 0); gpsimd cannot access
PSUM; DVE ops need 32-aligned partition starts; gpsimd tensor ops run
~5-10x slower than DVE.
"""

import math
import os
from contextlib import ExitStack

import numpy as np

B, S, C = 8, 1024, 1024
H, D = 16, 64
N_CORES = 8
KC = C // 128  # 8 contraction chunks of 128
NPAIR = H // 2

_CACHE = {}


def _cs_table():
    # Matches reference.rope_cos_sin computed in float32, transposed, with the
    # rotate-half sign folded into the sin half (rows 0-31 negated).
    f = np.float32
    inv = np.exp(np.arange(0, D, 2, dtype=f) * f(-(math.log(10000.0) / D))).astype(f)
    pos = np.arange(S, dtype=f)[:, None]
    ang = (pos * inv[None, :]).astype(f)  # (S, 32)
    ang = np.concatenate([ang, ang], axis=1)  # (S, 64)
    cosT = np.cos(ang).T.astype(f)  # (64, S)
    sinT = np.sin(ang).T.astype(f)
    sign = np.where(np.arange(D) < D // 2, f(-1.0), f(1.0))[:, None].astype(f)
    half = np.concatenate([cosT, sinT * sign], axis=1)  # (64, 2S)
    return np.concatenate([half, half], axis=0).astype(f)  # (128, 2S)


def declare_io(nc):
    from concourse import mybir

    f32 = mybir.dt.float32
    bf16 = mybir.dt.bfloat16
    return {
        "xT": nc.dram_tensor("xT", [C, S], bf16, kind="ExternalInput").ap(),
        "Wqk": nc.dram_tensor("Wqk", [C, 2 * C], bf16, kind="ExternalInput").ap(),
        "bqk": nc.dram_tensor("bqk", [128, 16], f32, kind="ExternalInput").ap(),
        "Wv": nc.dram_tensor("Wv", [C, C], bf16, kind="ExternalInput").ap(),
        "Wp": nc.dram_tensor("Wp", [C, C], bf16, kind="ExternalInput").ap(),
        "cs": nc.dram_tensor("cs", [128, 2 * S], bf16, kind="ExternalInput").ap(),
        "beff": nc.dram_tensor("beff", [1, C], f32, kind="ExternalInput").ap(),
        "out": nc.dram_tensor("out", [S, C], bf16, kind="ExternalOutput").ap(),
    }


def _emit(tc, io=None):
    from concourse import mybir
    from concourse.bass import ds, ts

    nc = tc.nc
    f32 = mybir.dt.float32
    bf16 = mybir.dt.bfloat16
    AF = mybir.ActivationFunctionType
    MUL = mybir.AluOpType.mult
    ADD = mybir.AluOpType.add

    if io is None:
        io = declare_io(nc)
    xT = io["xT"]
    Wqk = io["Wqk"]
    bqk = io["bqk"]
    Wv = io["Wv"]
    Wp = io["Wp"]
    cs = io["cs"]
    beff = io["beff"]
    out = io["out"]

    with ExitStack() as ctx:
        # ---------------- long-lived consts (right side) ----------------
        kons = ctx.enter_context(tc.tile_pool(name="kons", bufs=1, side="right"))
        cs_t = kons.tile([128, 2 * S], bf16, name="cs_t")
        bqk2 = kons.tile([128, 16], f32, name="bqk2")
        beff_r = kons.tile([1, C], f32, name="beff_r")
        beff_b = kons.tile([128, C], f32, name="beff_b")
        ones64 = kons.tile([1, 64], f32, name="ones64")
        nc.vector.memset(ones64[:], 1.0)

        # PSUM (8 banks): "sc" 3x[128,1024] (6) + "sm" 1x[128,512] (1)
        # + "pv" 1x[65,512] (1)
        ps_p = ctx.enter_context(tc.tile_pool(name="ps", bufs=1, space="PSUM"))

        def ps_sc(name):
            return ps_p.tile([128, S], f32, name=name, tag="sc", bufs=3)

        def ps_half(name, tag):
            return ps_p.tile([128, 512], f32, name=name, tag=tag, bufs=1)

        # the two 1-bank slots serve qk halves, v halves, and pv quarters
        # in strict emission-order alternation, so each bank gets ~2 lumps
        # of slack before its WAR reuse (rides out DVE queue delay on the
        # evac copies)
        _smp = {"i": 0}

        def next_small_tag():
            _smp["i"] ^= 1
            return "pv" if _smp["i"] else "sm"

        # ---------------- SBUF pools ----------------
        xk_p = ctx.enter_context(tc.tile_pool(name="xk", bufs=8))
        wqk_p = ctx.enter_context(tc.tile_pool(name="wqk", bufs=2))
        rr_p = ctx.enter_context(tc.tile_pool(name="rr", bufs=2))
        tm_p = ctx.enter_context(tc.tile_pool(name="tm", bufs=2))
        qkr_p = ctx.enter_context(tc.tile_pool(name="qkr", bufs=6))
        vst_p = ctx.enter_context(tc.tile_pool(name="vst", bufs=8))
        wv_p = ctx.enter_context(tc.tile_pool(name="wv", bufs=8))
        pT_p = ctx.enter_context(tc.tile_pool(name="pT", bufs=24))
        rec_p = ctx.enter_context(tc.tile_pool(name="rec", bufs=1))
        rb_p = ctx.enter_context(tc.tile_pool(name="rb", bufs=1))
        oS_p = ctx.enter_context(tc.tile_pool(name="oS", bufs=2))
        aT_p = ctx.enter_context(tc.tile_pool(name="aT", bufs=8))

        # -------- input DMAs: weights on gpsimd queue, rest on sync --------
        def qk_pair_weights(pair):
            # two [128,1024] half-DMAs, q-half first: the first qk matmul
            # then waits on 256KB instead of the full pair tile
            w = wqk_p.tile([128, 8 * 256], bf16, name=f"wqk{pair}", tag="wqk")
            for a in range(2):
                nc.gpsimd.dma_start(
                    out=w[:, ds(a * 1024, 1024)],
                    in_=Wqk[ts(pair, 128), ds(a * 1024, 1024)],
                )
            return w

        w_d = {0: qk_pair_weights(0)}
        xk = []
        for k in range(KC):
            t = xk_p.tile([128, S], bf16, name=f"xk{k}", tag="xk")
            xk.append(t)
            # n0 column halves first: the first qk pass reads only those,
            # so it gates on 1MB instead of 2MB of xT
            nc.sync.dma_start(out=t[:, 0:512], in_=xT[ts(k, 128), 0:512])
        nc.sync.dma_start(out=cs_t[:], in_=cs[:])
        nc.sync.dma_start(out=bqk2[:], in_=bqk[:])
        nc.sync.dma_start(out=beff_r[:], in_=beff[:])
        w_d[1] = qk_pair_weights(1)
        for k in range(KC):
            nc.gpsimd.dma_start(
                out=xk[k][:, 512:1024], in_=xT[ts(k, 128), 512:1024]
            )

        # qk GEMM in two [128,512] half-passes + RoPE chain
        rr_d = {}

        def qk_half(pair, a, n, ptag=None):
            gm = a * 8 + pair
            if (pair, a) not in rr_d:
                rr_d[(pair, a)] = rr_p.tile(
                    [128, 2 * S], bf16, name=f"rr{gm}", tag="rr"
                )
            rr = rr_d[(pair, a)]
            ps = ps_half(f"qps{gm}_{n}", next_small_tag())
            wts = w_d[pair]
            for k in range(KC):
                w = wts[:, a * 1024 + k * 128 : a * 1024 + k * 128 + 128]
                nc.tensor.matmul(
                    ps[:],
                    w,
                    xk[k][:, ds(n * 512, 512)],
                    start=(k == 0),
                    stop=(k == KC - 1),
                )
            nc.vector.tensor_scalar_add(
                rr[:, ds(n * 512, 512)], ps[:], bqk2[:, gm : gm + 1]
            )

        def qk_finish(pair, a):
            """rotate-half + cos/sin multiply-add -> RoPE'd qT or kT."""
            gm = a * 8 + pair
            rr = rr_d.pop((pair, a))
            for d0, s0 in ((0, 32), (32, 0), (64, 96), (96, 64)):
                nc.sync.dma_start(
                    out=rr[d0 : d0 + 32, S : 2 * S], in_=rr[s0 : s0 + 32, 0:S]
                )
            tm = tm_p.tile([128, 2 * S], bf16, name=f"tm{gm}", tag="tm")
            for half in range(2):
                nc.vector.tensor_tensor(
                    tm[:, ds(half * S, S)],
                    rr[:, ds(half * S, S)],
                    cs_t[:, ds(half * S, S)],
                    MUL,
                )
            qt = qkr_p.tile([128, S], bf16, name=f"qkr{gm}", tag="qkr")
            nc.vector.tensor_tensor(qt[:], tm[:, 0:S], tm[:, S : 2 * S], ADD)
            return qt

        # ---------------- v phase ----------------
        # wv tiles are declared here but their DMAs are issued after the
        # prologue qk emission on the sync queue: behind the rotate-half
        # DMAs (which wait on the RoPE chain), so the 2MB of Wv traffic
        # defers until the startup-critical xk/w0/w1 transfers are done.
        wv = []
        for k in range(KC):
            t = wv_p.tile([128, C], bf16, name=f"wv{k}", tag="wv")
            wv.append(t)
        vst = []

        def v_tile(mv):
            t = vst_p.tile([128, H * 65], bf16, name=f"vst{mv}", tag="vst")
            vst.append(t)
            hv = t[:].rearrange("p (h u) -> p h u", u=65)
            nc.vector.memset(hv[:, :, 64:65], 1.0)
            return hv

        def v_half(mv, n, hv, ptag=None):
            ps = ps_half(f"vps{mv}_{n}", next_small_tag())
            for k in range(KC):
                nc.tensor.matmul(
                    ps[:],
                    xk[k][:, ts(mv, 128)],
                    wv[k][:, ds(n * 512, 512)],
                    start=(k == 0),
                    stop=(k == KC - 1),
                )
            # channels n*512..: heads n*8..n*8+7 (head h = channel // 64).
            # Evac on DVE (light in iter 0): on the ACT queue these copies
            # would sit ahead of the pair-0 exps and delay pv(0) by ~9us.
            nc.vector.tensor_copy(hv[:, ds(n * 8, 8), 0:64], ps[:])

        # ---------------- attention helpers ----------------
        def sc_step(pair, sk, qt, kt, pTs):
            """Scores + exp for both heads of the pair at k-chunk sk.
            Even/odd heads sit on PE row groups 0-63 / 64-127 and co-run."""
            heads = (2 * pair, 2 * pair + 1)
            sc = {}
            for h in heads:
                pTs[(h, sk)] = pT_p.tile(
                    [128, S], bf16, name=f"pT{h}_{sk}", tag="pT"
                )
                sc[h] = ps_sc(f"sc{h}_{sk}")
            for n in range(2):
                for h in heads:
                    u = h % 2
                    nc.tensor.matmul(
                        sc[h][:, ds(n * 512, 512)],
                        kt[ds(64 * u, 64), ts(sk, 128)],
                        qt[ds(64 * u, 64), ds(n * 512, 512)],
                        start=True,
                        stop=True,
                    )
            for h in heads:
                nc.scalar.activation(pTs[(h, sk)][:], sc[h][:], AF.Exp, scale=0.125)

        def pv_quarter(pair, u, n, pTs, oS):
            """All 8 k-chunks of one (head-parity, n-half) quarter through
            one pv PSUM bank; a single [65,512] copy (row 64 = softmax
            denominator) releases the bank."""
            h = 2 * pair + u
            ps = ps_p.tile(
                [65, 512], f32, name=f"pv{h}_{n}", tag=next_small_tag(), bufs=1
            )
            for sk in range(KC):
                nc.tensor.matmul(
                    ps[:],
                    vst[sk][:, 65 * h : 65 * h + 65],
                    pTs[(h, sk)][:, ds(n * 512, 512)],
                    start=(sk == 0),
                    stop=(sk == KC - 1),
                )
            nc.vector.tensor_copy(oS[u][0:65, ds(n * 512, 512)], ps[0:65, :])

        # finish_pair is split in two emission phases so the DVE mults never
        # sit in the queue waiting on the (slow, ~2us) gpsimd broadcasts and
        # block the latency-critical pv-bank evac copies behind them.
        # reciprocal_approx_fast (~18 bits, plenty for softmax denominators)
        # reads the denominator row straight out of oS and writes partition
        # 0, where partition_broadcast needs it.
        def finish_pair_a(pair, oS):
            rbs = {}
            for u in range(2):
                # custom-DVE ops (like partition_broadcast) mishandle nonzero
                # partition offsets, so the denominator row is staged to
                # partition 0 by a sync-queue DMA (off the busy DVE)
                dn = rec_p.tile([1, S], f32, name=f"dn{pair}_{u}", tag=f"dn{u}")
                nc.sync.dma_start(out=dn[0:1, :], in_=oS[u][64:65, :])
                rec = rec_p.tile([1, S], f32, name=f"rec{pair}_{u}", tag=f"rc{u}")
                for nh in range(2):
                    nc.vector.reciprocal_approx_fast(
                        out=rec[0:1, ds(nh * 512, 512)],
                        in_=dn[0:1, ds(nh * 512, 512)],
                    )
                rb = rb_p.tile([64, S], f32, name=f"rb{pair}_{u}", tag=f"rb{u}")
                nc.gpsimd.partition_broadcast(rb[:], rec[0:1, :])
                rbs[u] = rb
            return rbs

        def finish_pair_b(pair, oS, rbs):
            aT_t = aT_p.tile([128, S], bf16, name=f"aT{pair}", tag="aT")
            for u in range(2):
                for nh in range(2):
                    nc.vector.tensor_tensor(
                        aT_t[ds(64 * u, 64), ds(nh * 512, 512)],
                        oS[u][0:64, ds(nh * 512, 512)],
                        rbs[u][:, ds(nh * 512, 512)],
                        MUL,
                    )
            return aT_t

        aT = []
        qk_done = {}
        QORDER = [(0, 0), (0, 1), (1, 0), (1, 1)]  # (head parity, n-half)

        # ================= prologue: qk(0), qk(1) =================
        for pair in (0, 1):
            for a in (0, 1):
                qk_half(pair, a, 0, "sm")
                qk_half(pair, a, 1, "pv")
                qk_done.setdefault(pair, []).append(qk_finish(pair, a))
        nc.gpsimd.partition_broadcast(beff_b[:], beff_r[0:1, :])
        for k in range(KC):
            nc.sync.dma_start(out=wv[k][:], in_=Wv[ts(k, 128), :])

        # ================= iter 0: scores(0) + v GEMM + qk(2) ============
        pTs_prev = {}
        w_d[2] = qk_pair_weights(2)
        for sk in range(KC):
            sc_step(0, sk, qk_done[0][0], qk_done[0][1], pTs_prev)
            hv = v_tile(sk)
            v_half(sk, 0, hv, "sm")
            v_half(sk, 1, hv, "pv")
            if sk % 2 == 1:
                idx = sk // 2  # 0..3 -> (a, n)
                a, n = idx // 2, idx % 2
                qk_half(2, a, n, "sm")
                if n == 1:
                    qk_done.setdefault(2, []).append(qk_finish(2, a))
        w_d[3] = qk_pair_weights(3)

        den_d, oS_d, rbs_d, aTq = {}, {}, {}, []

        # ================= steady iterations p = 1..7 =================
        # iteration p: scores(p) + pv quarters (p-1) + qk halves (p+2)
        for p in range(1, NPAIR):
            pTs = {}
            qt, kt = qk_done[p]
            oS = {
                u: oS_p.tile([65, S], f32, name=f"oS{p - 1}_{u}", tag=f"oS{u}")
                for u in range(2)
            }
            # qk halves for pairs 3..7 spread 4/4/4/4/2/2 over iters 1..6
            # (pair 7's kt finishes RoPE just before iter 7 consumes it), so
            # the late iterations keep PE work while the ACT catches up on
            # exp; halves sit at sk 0,2,4,7 and pv quarters at 1,3,5,6 (the
            # last quarter must not sit adjacent to the next iteration's
            # first quarter in the 2-bank rotation)
            qk_sched = {
                1: [0, 1, 2, 3], 2: [4, 5, 6, 7], 3: [8, 9, 10, 11],
                4: [12, 13, 14, 15], 5: [16, 17], 6: [18, 19],
            }
            sk_pos = (0, 2, 4, 7)
            qk_at = {
                sk_pos[i]: g for i, g in enumerate(qk_sched.get(p, []))
            }
            pv_at = {1: 0, 3: 1, 5: 2, 6: 3}
            for sk in range(KC):
                sc_step(p, sk, qt, kt, pTs)
                if sk in qk_at:
                    g = qk_at[sk]
                    q, idx = 3 + g // 4, g % 4
                    a, n = idx // 2, idx % 2
                    qk_half(q, a, n)
                    if n == 1:
                        qk_done.setdefault(q, []).append(qk_finish(q, a))
                if sk in pv_at:
                    u, n = QORDER[pv_at[sk]]
                    pv_quarter(p - 1, u, n, pTs_prev, oS)
                if sk == 3 and aTq:
                    pp2 = aTq[0]  # normalize pair p-2 (evac'd last iter)
                    rbs_d[pp2] = finish_pair_a(pp2, oS_d[pp2])
                if sk == 7 and aTq:
                    pp2 = aTq.pop(0)
                    aT.append(finish_pair_b(pp2, oS_d[pp2], rbs_d.pop(pp2)))
            oS_d[p - 1] = oS
            aTq.append(p - 1)
            if p + 3 < NPAIR:
                w_d[p + 3] = qk_pair_weights(p + 3)
            if p == 4:
                # proj weights reuse the wv slots (v GEMM done in iter 0)
                wp = []
                for k in range(KC):
                    t = wv_p.tile([128, C], bf16, name=f"wp{k}", tag="wv")
                    nc.gpsimd.dma_start(out=t[:], in_=Wp[ts(k, 128), :])
                    wp.append(t)
            pTs_prev = pTs

        # ================= epilogue: pv(7) + projection =================
        oS7 = {
            u: oS_p.tile([65, S], f32, name=f"oS7_{u}", tag=f"oS{u}")
            for u in range(2)
        }
        p6 = aTq.pop(0)
        aT6 = aT_p.tile([128, S], bf16, name="aT6", tag="aT")
        recE = {
            (pr, u): rec_p.tile([1, S], f32, name=f"recE{pr}_{u}", tag=f"rc{u}")
            for pr in (6, 7)
            for u in range(2)
        }

        def finishE(pr, oS, aT_t, nh):
            """PE outer-product normalize for one n-half (epilogue pairs:
            the small PSUM banks are free and the chain must be short)."""
            h = ds(nh * 512, 512)
            for u in range(2):
                dn = rec_p.tile(
                    [1, S], f32, name=f"dnE{pr}_{u}_{nh}", tag=f"dn{u}"
                )
                nc.sync.dma_start(out=dn[0:1, h], in_=oS[u][64:65, h])
                rec = recE[(pr, u)]
                nc.vector.reciprocal_approx_fast(out=rec[0:1, h], in_=dn[0:1, h])
                rbp = ps_p.tile(
                    [64, 512], f32, name=f"rbE{u}_{nh}", tag=next_small_tag(), bufs=1
                )
                nc.tensor.matmul(rbp[:], ones64[:], rec[0:1, h], start=True, stop=True)
                nc.vector.tensor_tensor(
                    aT_t[ds(64 * u, 64), h], oS[u][0:64, h], rbp[:], MUL
                )

        finishE(p6, oS_d[p6], aT6, 0)
        aT.append(aT6)  # aT[6]

        def proj_open(m):
            # contracts pairs 0..6 (everything that does not depend on the
            # last pair's aT) into an sc slot
            pp = ps_sc(f"pp{m}")
            for k in range(KC - 1):
                for n in range(2):
                    nc.tensor.matmul(
                        pp[:, ds(n * 512, 512)],
                        aT[k][:, ts(m, 128)],
                        wp[k][:, ds(n * 512, 512)],
                        start=(k == 0),
                        stop=False,
                    )
            return pp

        def proj_close(m, pp):
            k = KC - 1
            for n in range(2):
                nc.tensor.matmul(
                    pp[:, ds(n * 512, 512)],
                    aT[k][:, ts(m, 128)],
                    wp[k][:, ds(n * 512, 512)],
                    start=False,
                    stop=True,
                )
            # bias add (b_v @ W_proj + b_proj) during the PSUM evac; the ob
            # staging tile reuses a wqk slot (qk weights are done)
            # bf16 staging halves the output DMA (the host upcasts); the
            # rounding adds ~0.2% rel err, well inside the gate
            ob = wqk_p.tile([128, C], bf16, name=f"ob{m}", tag="wqk")
            nc.vector.tensor_tensor(ob[:], pp[:], beff_b[:], ADD)
            q = nc.sync if m % 2 == 0 else nc.gpsimd
            q.dma_start(out=out[ts(m, 128), :], in_=ob[:])

        # pair-7 finish is pipelined per n-half: the rb broadcast runs as a
        # PE outer product (ones64 x rec-row) into the free sm/pv PSUM
        # slots, and closes 0-3 need only the n0 half of aT7, so the
        # projection never waits on the full normalize chain.
        aT7 = aT_p.tile([128, S], bf16, name="aT7", tag="aT")

        pv_quarter(NPAIR - 1, 0, 0, pTs_prev, oS7)
        finishE(p6, oS_d[p6], aT6, 1)
        pv_quarter(NPAIR - 1, 0, 1, pTs_prev, oS7)
        pps = {0: proj_open(0)}
        pv_quarter(NPAIR - 1, 1, 0, pTs_prev, oS7)
        pps[1] = proj_open(1)
        pv_quarter(NPAIR - 1, 1, 1, pTs_prev, oS7)
        finishE(NPAIR - 1, oS7, aT7, 0)
        aT.append(aT7)  # aT[7]
        pps[2] = proj_open(2)
        finishE(NPAIR - 1, oS7, aT7, 1)
        for m in range(3, S // 128):
            proj_close(m - 3, pps.pop(m - 3))
            pps[m] = proj_open(m)
        for m in range(S // 128 - 3, S // 128):
            proj_close(m, pps.pop(m))


def build_program():
    """Build + compile the Bass program (cached)."""
    if "nc" in _CACHE:
        return _CACHE["nc"]
    import concourse.tile as tile
    from concourse import bacc

    nc = bacc.Bacc(
        "TRN2", target_bir_lowering=False, debug=False, num_devices=N_CORES
    )
    with tile.TileContext(nc) as tc:
        _emit(tc)
    nc.compile()
    _CACHE["nc"] = nc
    return nc


def host_inputs(x, W_qkv, b_qkv, W_proj, b_proj):
    """Per-core input maps (host-side shard + layout prep)."""
    import ml_dtypes

    f = np.float32
    bf = ml_dtypes.bfloat16
    x = np.asarray(x, dtype=f)
    W_qkv = np.asarray(W_qkv, dtype=f)
    b_qkv = np.asarray(b_qkv, dtype=f)
    W_proj = np.asarray(W_proj, dtype=f)
    b_proj = np.asarray(b_proj, dtype=f)
    Wq4 = W_qkv[:, : 2 * C].reshape(8, 128, 2, 8, 128)  # (k, p, a, g, c)
    Wqk = np.ascontiguousarray(
        Wq4.transpose(3, 1, 2, 0, 4).reshape(8 * 128, 2 * C)
    ).astype(bf)  # (g*128+p, a*1024+k*128+c)
    bqk = np.ascontiguousarray(b_qkv[: 2 * C].reshape(16, 128).T).astype(f)
    Wv = W_qkv[:, 2 * C :].astype(bf)
    Wp = W_proj.astype(bf)
    # softmax rows sum to 1: attn(v + b_v) = attn(v) + b_v, so the v bias
    # rides through attention and folds into the proj bias.
    beff = (b_qkv[None, 2 * C :] @ W_proj + b_proj[None, :]).astype(f)
    cs = _cs_table().astype(bf)
    maps = []
    for b in range(B):
        maps.append(
            {
                "xT": np.ascontiguousarray(x[b].T).astype(bf),
                "Wqk": Wqk,
                "bqk": bqk,
                "Wv": Wv,
                "Wp": Wp,
                "cs": cs,
                "beff": beff,
            }
        )
    return maps


def _install_neff_cache():
    """Memoize the BIR->NEFF compile so repeat kernel() calls skip the
    multi-minute neuronxcc invocation (pure caching, same artifacts)."""
    if _CACHE.get("neff_cache"):
        return
    import hashlib
    import shutil
    import tempfile

    import concourse.bass2jax as b2j
    import concourse.bass_utils as bu

    cache_dir = os.path.join(tempfile.gettempdir(), "bass_neff_cache")
    os.makedirs(cache_dir, exist_ok=True)
    orig = bu.compile_bir_kernel

    def cached(bir_json, tmpdir, neff_name="file.neff"):
        raw = bir_json if isinstance(bir_json, bytes) else bir_json.encode()
        hit = os.path.join(cache_dir, hashlib.sha256(raw).hexdigest() + ".neff")
        if os.path.exists(hit):
            dst = os.path.join(tmpdir, neff_name)
            shutil.copyfile(hit, dst)
            return dst
        path = orig(bir_json, tmpdir, neff_name)
        try:
            shutil.copyfile(path, hit)
        except OSError:
            pass
        return path

    bu.compile_bir_kernel = cached
    b2j.compile_bir_kernel = cached
    _CACHE["neff_cache"] = True


def kernel(x, W_qkv, b_qkv, W_proj, b_proj):
    from concourse.bass_utils import run_bass_kernel_spmd

    _install_neff_cache()
    nc = build_program()
    in_maps = host_inputs(x, W_qkv, b_qkv, W_proj, b_proj)
    res = run_bass_kernel_spmd(nc, in_maps, list(range(N_CORES)))
    return np.stack(
        [np.asarray(r["out"]) for r in res.results], axis=0
    ).astype(np.float32)


if __name__ == "__main__":
    nc = build_program()
    print("program built + compiled OK")
